# revision 25
# baseline (speedup 1.0000x reference)
"""Newton-SOR batched solver for Trainium2, 8 NeuronCores, data parallel.

Math: the reference's while-loop always runs all MAXITER=16 iterations
(the fp32 residual-norm floor ~5e-5 never reaches TOL=1e-6), and the
iterate converges to the fixed point F(x*)=0, so an approximate-but-
convergent inner solve reproduces the reference to ~1e-5 relative.

Per outer iteration (damped Newton-Jacobi, K=1 Neumann):
    d~ = diag(A) + 3 x^2
    v  = omega * F / d~      (rounded to bf16; the update uses the SAME
                              rounded vector, keeping F exactly consistent)
    x' = x - v
    F' = F - A @ v + (x'^3 - x^3)

The heavy op is 2048 independent 128x128 matvecs per iteration. They run
on TensorE as bf16 self-loading matmuls (N=1) with fp32 PSUM accumulation
(~32ns/element steady): A = A1 + A2, both bf16; F is carried with
A1-only applies and the *exactly linear* deferred part A2 @ (sum(v)-x0)
is folded in at a few correction iterations (drift contracts afterwards).
Everything stays in transposed layout [var, element] so TensorE needs no
transposes; VectorE/ScalarE pointwise work is hoisted off the PSUM
critical path so it hides under TensorE's stream. The 16th iteration
needs no matvec at all (F_16 is never consumed).
"""

import numpy as np
import ml_dtypes

BATCH = 2048
N = 128
NCORES = 8
PER_CORE = BATCH // NCORES          # 256
NTILES = 2                          # halves of 128 elements each
TPE = PER_CORE // NTILES            # 128 elements per tile
NITER = 16
# Elements are globally sorted by omega: tile0 gets the slow-converging
# (low omega) half and runs 15 applies; tile1 gets the fast half and
# needs only 10 (validated: total rel err ~6e-6 either way).
NAPPLY_T = (15, 10)
CORR_T = (frozenset({8, 15}), frozenset({7, 10}))
RECIP_FULL = 2                      # full reciprocal for k <= this
RECIP_NEWTON = 5                    # one Newton refresh for k <= this
NCHUNK = 16                         # DMA chunks per A1 tile
NHALF = 2                           # column-halves for PSUM critical path
HTPE = TPE // NHALF

_BF16 = ml_dtypes.bfloat16

_compiled = None


def _build():
    import concourse.bacc as bacc
    import concourse.mybir as mybir
    from concourse.tile import TileContext

    f32 = mybir.dt.float32
    bf16 = mybir.dt.bfloat16
    op = mybir.AluOpType

    nc = bacc.Bacc("TRN2", target_bir_lowering=False, debug=False)

    at1 = [
        nc.dram_tensor(f"at1_{t}", [N, TPE * N], bf16, kind="ExternalInput")
        for t in range(NTILES)
    ]
    at2 = [
        nc.dram_tensor(f"at2_{t}", [N, TPE * N], bf16, kind="ExternalInput")
        for t in range(NTILES)
    ]
    x0_d = nc.dram_tensor("x0t", [N, PER_CORE], f32, kind="ExternalInput")
    b_d = nc.dram_tensor("bt", [N, PER_CORE], f32, kind="ExternalInput")
    da_d = nc.dram_tensor("dat", [N, PER_CORE], f32, kind="ExternalInput")
    om_d = nc.dram_tensor("omt", [N, PER_CORE], f32, kind="ExternalInput")
    out_d = nc.dram_tensor("outt", [N, PER_CORE], f32, kind="ExternalOutput")

    with TileContext(nc) as tc:
        with (
            tc.tile_pool(name="wts", bufs=1) as wts,
            tc.tile_pool(name="vec", bufs=1) as vec,
            tc.tile_pool(name="roll", bufs=3) as roll,
            tc.tile_pool(name="ps", bufs=2, space="PSUM") as psp,
        ):
            # small vectors first so pointwise prep can start immediately
            x0_sb = vec.tile([N, PER_CORE], f32, name="x0sb")
            nc.sync.dma_start(x0_sb[:, :], x0_d[:, :])
            b_sb = vec.tile([N, PER_CORE], f32, name="bsb")
            nc.sync.dma_start(b_sb[:, :], b_d[:, :])
            da_sb = vec.tile([N, PER_CORE], f32, name="dasb")
            nc.sync.dma_start(da_sb[:, :], da_d[:, :])
            om_sb = vec.tile([N, PER_CORE], f32, name="omsb")
            nc.sync.dma_start(om_sb[:, :], om_d[:, :])

            # Bulk weights go on the gpsimd SWDGE queue (~250GB/s measured;
            # the sync HWDGE queue trickles at ~50GB/s, so it only carries
            # the small vectors above and the last-needed A2 tile).
            a1_sb = []
            for t in range(NTILES):
                a1_t = wts.tile([N, TPE * N], bf16, name=f"a1sb{t}", tag=f"a1{t}")
                a1_sb.append(a1_t)
            csz = TPE * N // NCHUNK
            for t in range(NTILES):
                for q in range(NCHUNK):
                    nc.gpsimd.dma_start(
                        a1_sb[t][:, q * csz : (q + 1) * csz],
                        at1[t][:, q * csz : (q + 1) * csz],
                    )
            a2_sb = []
            for t in range(NTILES):
                a2_t = wts.tile([N, TPE * N], bf16, name=f"a2sb{t}", tag=f"a2{t}")
                nc.gpsimd.dma_start(a2_t[:, :], at2[t][:, :])
                a2_sb.append(a2_t)

            def apply_mms(ps, a_sb, v_bf, e0=0, e1=TPE):
                for e in range(e0, e1):
                    nc.tensor.matmul(
                        ps[:, e : e + 1],
                        a_sb[:, e * N : (e + 1) * N],
                        v_bf[:, e : e + 1],
                        start=True,
                        stop=True,
                    )

            # per-tile persistent state
            F_t = [vec.tile([N, TPE], f32, name=f"F{t}") for t in range(2)]
            wa_t = [vec.tile([N, TPE], f32, name=f"wa{t}") for t in range(2)]
            r_t = [vec.tile([N, TPE], f32, name=f"r{t}") for t in range(2)]
            s_t = [vec.tile([N, TPE], f32, name=f"s{t}") for t in range(2)]
            x_t = [None] * NTILES
            x3_t = [None] * NTILES
            v_t = [None] * NTILES
            vb_t = [None] * NTILES

            # ---- init per tile ----
            def emit_init(t):
                cs = slice(t * TPE, (t + 1) * TPE)
                xb = roll.tile([N, TPE], bf16, name=f"xb{t}", tag=f"vb{t}")
                nc.scalar.copy(xb[:, :], x0_sb[:, cs])
                x = roll.tile([N, TPE], f32, name=f"x{t}", tag=f"x{t}")
                nc.scalar.copy(x[:, :], xb[:, :])          # x = round(x0)
                ps = psp.tile([N, TPE], f32, name=f"psi{t}", tag=f"ps{t}")
                apply_mms(ps, a1_sb[t], xb)
                # hoisted: everything except the PSUM merge
                nc.vector.tensor_scalar_mul(wa_t[t][:, :], x[:, :], -1.0)
                x2 = roll.tile([N, TPE], f32, name=f"x2{t}", tag=f"x2{t}")
                nc.scalar.square(x2[:, :], x[:, :])
                x3 = roll.tile([N, TPE], f32, name=f"x3{t}", tag=f"x3{t}")
                nc.vector.tensor_mul(x3[:, :], x2[:, :], x[:, :])
                dt_ = roll.tile([N, TPE], f32, name=f"dt{t}", tag=f"dt{t}")
                nc.vector.scalar_tensor_tensor(
                    dt_[:, :], x2[:, :], 3.0, da_sb[:, cs],
                    op0=op.mult, op1=op.add,
                )
                nc.vector.reciprocal(r_t[t][:, :], dt_[:, :])
                nc.vector.tensor_mul(s_t[t][:, :], r_t[t][:, :], om_sb[:, cs])
                nc.vector.tensor_sub(F_t[t][:, :], x3[:, :], b_sb[:, cs])
                # PSUM merge + v_1, per column-half for pipelining
                v_bf = roll.tile([N, TPE], bf16, name=f"vb{t}", tag=f"vb{t}")
                for h in range(NHALF):
                    hs = slice(h * HTPE, (h + 1) * HTPE)
                    nc.vector.tensor_add(
                        F_t[t][:, hs], F_t[t][:, hs], ps[:, hs]
                    )
                    nc.vector.tensor_mul(
                        v_bf[:, hs], F_t[t][:, hs], s_t[t][:, hs]
                    )
                x_t[t], x3_t[t], vb_t[t] = x, x3, v_bf

            # ---- one iteration (last one per tile needs no apply) ----
            def emit_iter(k, t):
                if True:
                    corr = k in CORR_T[t]
                    cs = slice(t * TPE, (t + 1) * TPE)
                    x, x3, v_bf = x_t[t], x3_t[t], vb_t[t]
                    F, wa, r, s = F_t[t], wa_t[t], r_t[t], s_t[t]

                    ps = psp.tile([N, TPE], f32, name=f"psk{t}_{k}", tag=f"ps{t}")
                    ps2 = None
                    w32 = w_bf = None
                    if corr:
                        # w-chain first so the A2 matmuls aren't starved
                        w32 = roll.tile([N, TPE], f32, name=f"w{t}_{k}", tag=f"w{t}")
                        nc.vector.tensor_add(w32[:, :], wa[:, :], v_bf[:, :])
                        w_bf = roll.tile(
                            [N, TPE], bf16, name=f"wb{t}_{k}", tag=f"wb{t}"
                        )
                        nc.scalar.copy(w_bf[:, :], w32[:, :])
                        ps2 = psp.tile(
                            [N, TPE], f32, name=f"psc{t}_{k}", tag=f"pc{t}"
                        )
                    apply_mms(ps, a1_sb[t], v_bf)
                    if corr:
                        apply_mms(ps2, a2_sb[t], w_bf)

                    # --- hoisted pointwise (runs under the PE stream) ---
                    xn = roll.tile([N, TPE], f32, name=f"x{t}_{k}", tag=f"x{t}")
                    nc.vector.tensor_sub(xn[:, :], x[:, :], v_bf[:, :])
                    x2 = roll.tile([N, TPE], f32, name=f"x2{t}_{k}", tag=f"x2{t}")
                    nc.scalar.square(x2[:, :], xn[:, :])
                    nx3 = roll.tile([N, TPE], f32, name=f"x3{t}_{k}", tag=f"x3{t}")
                    nc.vector.tensor_mul(nx3[:, :], x2[:, :], xn[:, :])
                    dc = roll.tile([N, TPE], f32, name=f"dc{t}_{k}", tag=f"dt{t}")
                    nc.vector.tensor_sub(dc[:, :], nx3[:, :], x3[:, :])
                    nc.vector.tensor_add(F[:, :], F[:, :], dc[:, :])
                    if corr:
                        nc.vector.tensor_sub(wa[:, :], w32[:, :], w_bf[:, :])
                    else:
                        nc.gpsimd.tensor_add(wa[:, :], wa[:, :], v_bf[:, :])
                    # d~(x'), reciprocal policy, s
                    if k + 1 <= RECIP_NEWTON:
                        dt_ = roll.tile(
                            [N, TPE], f32, name=f"dt{t}_{k}", tag=f"dt{t}"
                        )
                        nc.vector.scalar_tensor_tensor(
                            dt_[:, :], x2[:, :], 3.0, da_sb[:, cs],
                            op0=op.mult, op1=op.add,
                        )
                        if k + 1 <= RECIP_FULL:
                            nc.vector.reciprocal(r[:, :], dt_[:, :])
                        else:
                            # r <- r*(2 - d*r)
                            tmp = roll.tile(
                                [N, TPE], f32, name=f"tm{t}_{k}", tag=f"tm{t}"
                            )
                            nc.vector.tensor_mul(tmp[:, :], dt_[:, :], r[:, :])
                            nc.vector.tensor_scalar(
                                tmp[:, :], tmp[:, :], -1.0, 2.0,
                                op0=op.mult, op1=op.add,
                            )
                            nc.vector.tensor_mul(r[:, :], r[:, :], tmp[:, :])
                        nc.vector.tensor_mul(s[:, :], r[:, :], om_sb[:, cs])

                    # --- PSUM critical path, pipelined per column-slice ---
                    # (quarters when the other tile has retired: less PE work
                    # per iteration to hide the chain under)
                    nsplit = 4 if k > min(NAPPLY_T) else NHALF
                    stpe = TPE // nsplit
                    vbn = roll.tile([N, TPE], bf16, name=f"vb{t}_{k}", tag=f"vb{t}")
                    for h in range(nsplit):
                        hs = slice(h * stpe, (h + 1) * stpe)
                        nc.vector.tensor_sub(F[:, hs], F[:, hs], ps[:, hs])
                        if ps2 is not None:
                            nc.vector.tensor_sub(F[:, hs], F[:, hs], ps2[:, hs])
                        nc.vector.tensor_mul(vbn[:, hs], F[:, hs], s[:, hs])

                    x_t[t], x3_t[t], vb_t[t] = xn, nx3, vbn

            # ---- final half-step + output ----
            def emit_final(t):
                cs = slice(t * TPE, (t + 1) * TPE)
                xn = roll.tile([N, TPE], f32, name=f"xf{t}", tag=f"x{t}")
                nc.vector.tensor_sub(xn[:, :], x_t[t][:, :], vb_t[t][:, :])
                nc.sync.dma_start(out_d[:, cs], xn[:, :])

            # Staggered emission: PE executes in strict program order, so
            # tile1 (whose weights arrive later) trails tile0 by one unit to
            # avoid head-of-line blocking during the load phase.
            units = {
                t: (
                    [("init", t)]
                    + [("iter", k, t) for k in range(1, NAPPLY_T[t] + 1)]
                    + [("final", t)]
                )
                for t in range(NTILES)
            }
            seq = []
            n0, n1 = len(units[0]), len(units[1])
            for i in range(max(n0, n1 + 1)):
                if i < n0:
                    seq.append(units[0][i])
                if 0 <= i - 1 < n1:
                    seq.append(units[1][i - 1])
            for u in seq:
                if u[0] == "init":
                    emit_init(u[1])
                elif u[0] == "iter":
                    emit_iter(u[1], u[2])
                else:
                    emit_final(u[1])

    nc.compile()
    return nc


def _get_compiled():
    global _compiled
    if _compiled is None:
        _compiled = _build()
    return _compiled


def _perm_for(omega):
    """Global omega sort: slow (low omega) half feeds every core's tile0,
    fast half feeds tile1. perm[slot] = source batch index."""
    order = np.argsort(np.asarray(omega, dtype=np.float32)[:, 0], kind="stable")
    half = BATCH // 2
    perm = np.empty(BATCH, dtype=np.int64)
    for c in range(NCORES):
        perm[c * PER_CORE : c * PER_CORE + TPE] = order[c * TPE : (c + 1) * TPE]
        perm[c * PER_CORE + TPE : (c + 1) * PER_CORE] = order[
            half + c * TPE : half + (c + 1) * TPE
        ]
    return perm


def _prep_inputs(x, A, b, omega, perm):
    """Host-side shard + layout prep. Returns list of per-core in_maps."""
    A = np.ascontiguousarray(A, dtype=np.float32)
    x = np.asarray(x, dtype=np.float32)[perm]
    b = np.asarray(b, dtype=np.float32)[perm]
    omega = np.asarray(omega, dtype=np.float32)[perm]

    Ap = A[perm]
    A1 = Ap.astype(_BF16)
    A2 = (Ap - A1.astype(np.float32)).astype(_BF16)
    dA = np.ascontiguousarray(np.diagonal(Ap, axis1=1, axis2=2))

    in_maps = []
    for c in range(NCORES):
        sl = slice(c * PER_CORE, (c + 1) * PER_CORE)
        m = {}
        for t in range(NTILES):
            ts = slice(c * PER_CORE + t * TPE, c * PER_CORE + (t + 1) * TPE)
            # lhsT layout [j, (e, i)]: element e's weights = A[e].T
            m[f"at1_{t}"] = np.ascontiguousarray(
                A1[ts].transpose(2, 0, 1)
            ).reshape(N, TPE * N)
            m[f"at2_{t}"] = np.ascontiguousarray(
                A2[ts].transpose(2, 0, 1)
            ).reshape(N, TPE * N)
        m["x0t"] = np.ascontiguousarray(x[sl].T)
        m["bt"] = np.ascontiguousarray(b[sl].T)
        m["dat"] = np.ascontiguousarray(dA[sl].T)
        m["omt"] = np.ascontiguousarray(
            np.broadcast_to(omega[sl].reshape(1, PER_CORE), (N, PER_CORE))
        )
        in_maps.append(m)
    return in_maps


def _run(inputs, trace=False):
    from concourse.bass_utils import run_bass_kernel_spmd

    nc = _get_compiled()
    perm = _perm_for(inputs["omega"])
    in_maps = _prep_inputs(
        inputs["x"], inputs["A"], inputs["b"], inputs["omega"], perm
    )
    res = run_bass_kernel_spmd(
        nc, in_maps, core_ids=list(range(NCORES)), trace=trace
    )
    out = np.empty((BATCH, N), dtype=np.float32)
    for c in range(NCORES):
        out[perm[c * PER_CORE : (c + 1) * PER_CORE]] = res.results[c]["outt"].T
    return out, res


def kernel(x, A, b, omega):
    out, _ = _run({"x": x, "A": A, "b": b, "omega": omega}, trace=False)
    return out


# revision 26
# speedup vs baseline: 1.0104x; 1.0104x over previous
"""Newton-SOR batched solver for Trainium2, 8 NeuronCores, data parallel.

Math: the reference's while-loop always runs all MAXITER=16 iterations
(the fp32 residual-norm floor ~5e-5 never reaches TOL=1e-6), and the
iterate converges to the fixed point F(x*)=0, so an approximate-but-
convergent inner solve reproduces the reference to ~1e-5 relative.

Per outer iteration (damped Newton-Jacobi, K=1 Neumann):
    d~ = diag(A) + 3 x^2
    v  = omega * F / d~      (rounded to bf16; the update uses the SAME
                              rounded vector, keeping F exactly consistent)
    x' = x - v
    F' = F - A @ v + (x'^3 - x^3)

The heavy op is 2048 independent 128x128 matvecs per iteration. They run
on TensorE as bf16 self-loading matmuls (N=1) with fp32 PSUM accumulation
(~32ns/element steady): A = A1 + A2, both bf16; F is carried with
A1-only applies and the *exactly linear* deferred part A2 @ (sum(v)-x0)
is folded in at a few correction iterations (drift contracts afterwards).
Everything stays in transposed layout [var, element] so TensorE needs no
transposes; VectorE/ScalarE pointwise work is hoisted off the PSUM
critical path so it hides under TensorE's stream. The 16th iteration
needs no matvec at all (F_16 is never consumed).
"""

import numpy as np
import ml_dtypes

BATCH = 2048
N = 128
NCORES = 8
PER_CORE = BATCH // NCORES          # 256
NTILES = 2                          # halves of 128 elements each
TPE = PER_CORE // NTILES            # 128 elements per tile
NITER = 16
# Elements are globally sorted by omega: tile0 gets the slow-converging
# (low omega) half and runs 15 applies; tile1 gets the fast half and
# needs only 10 (validated: total rel err ~6e-6 either way).
NAPPLY_T = (15, 10)
CORR_T = (frozenset({8, 15}), frozenset({7, 10}))
RECIP_FULL = 2                      # full reciprocal for k <= this
RECIP_NEWTON = 5                    # one Newton refresh for k <= this
NCHUNK = 16                         # DMA chunks per A1 tile
NHALF = 2                           # column-halves for PSUM critical path
HTPE = TPE // NHALF

_BF16 = ml_dtypes.bfloat16

_compiled = None


def _build():
    import concourse.bacc as bacc
    import concourse.mybir as mybir
    from concourse.tile import TileContext

    f32 = mybir.dt.float32
    bf16 = mybir.dt.bfloat16
    op = mybir.AluOpType

    nc = bacc.Bacc("TRN2", target_bir_lowering=False, debug=False)

    at1 = [
        nc.dram_tensor(f"at1_{t}", [N, TPE * N], bf16, kind="ExternalInput")
        for t in range(NTILES)
    ]
    at2 = [
        nc.dram_tensor(f"at2_{t}", [N, TPE * N], bf16, kind="ExternalInput")
        for t in range(NTILES)
    ]
    x0_d = nc.dram_tensor("x0t", [N, PER_CORE], f32, kind="ExternalInput")
    b_d = nc.dram_tensor("bt", [N, PER_CORE], f32, kind="ExternalInput")
    da_d = nc.dram_tensor("dat", [N, PER_CORE], f32, kind="ExternalInput")
    om_d = nc.dram_tensor("omt", [N, PER_CORE], f32, kind="ExternalInput")
    out_d = nc.dram_tensor("outt", [N, PER_CORE], f32, kind="ExternalOutput")

    with TileContext(nc) as tc:
        with (
            tc.tile_pool(name="wts", bufs=1) as wts,
            tc.tile_pool(name="vec", bufs=1) as vec,
            tc.tile_pool(name="roll", bufs=2) as roll,
            tc.tile_pool(name="ps", bufs=2, space="PSUM") as psp,
        ):
            # small vectors first so pointwise prep can start immediately
            x0_sb = vec.tile([N, PER_CORE], f32, name="x0sb")
            nc.sync.dma_start(x0_sb[:, :], x0_d[:, :])
            b_sb = vec.tile([N, PER_CORE], f32, name="bsb")
            nc.sync.dma_start(b_sb[:, :], b_d[:, :])
            da_sb = vec.tile([N, PER_CORE], f32, name="dasb")
            nc.sync.dma_start(da_sb[:, :], da_d[:, :])
            om_sb = vec.tile([N, PER_CORE], f32, name="omsb")
            nc.sync.dma_start(om_sb[:, :], om_d[:, :])

            # Bulk weights go on the gpsimd SWDGE queue (~250GB/s measured;
            # the sync HWDGE queue trickles at ~50GB/s, so it only carries
            # the small vectors above and the last-needed A2 tile).
            a1_sb = []
            for t in range(NTILES):
                a1_t = wts.tile([N, TPE * N], bf16, name=f"a1sb{t}", tag=f"a1{t}")
                a1_sb.append(a1_t)
            csz = TPE * N // NCHUNK
            for t in range(NTILES):
                for q in range(NCHUNK):
                    nc.gpsimd.dma_start(
                        a1_sb[t][:, q * csz : (q + 1) * csz],
                        at1[t][:, q * csz : (q + 1) * csz],
                    )
            a2_sb = []
            for t in range(NTILES):
                a2_t = wts.tile([N, TPE * N], bf16, name=f"a2sb{t}", tag=f"a2{t}")
                nc.gpsimd.dma_start(a2_t[:, :], at2[t][:, :])
                a2_sb.append(a2_t)

            def apply_mms(ps, a_sb, v_bf, e0=0, e1=TPE):
                for e in range(e0, e1):
                    nc.tensor.matmul(
                        ps[:, e : e + 1],
                        a_sb[:, e * N : (e + 1) * N],
                        v_bf[:, e : e + 1],
                        start=True,
                        stop=True,
                    )

            # per-tile persistent state
            F_t = [vec.tile([N, TPE], f32, name=f"F{t}") for t in range(2)]
            wa_t = [vec.tile([N, TPE], f32, name=f"wa{t}") for t in range(2)]
            r_t = [vec.tile([N, TPE], f32, name=f"r{t}") for t in range(2)]
            s_t = [vec.tile([N, TPE], f32, name=f"s{t}") for t in range(2)]
            x_t = [None] * NTILES
            x3_t = [None] * NTILES
            v_t = [None] * NTILES
            vb_t = [None] * NTILES

            # ---- init per tile ----
            def emit_init(t):
                cs = slice(t * TPE, (t + 1) * TPE)
                xb = roll.tile([N, TPE], bf16, name=f"xb{t}", tag=f"vb{t}")
                nc.scalar.copy(xb[:, :], x0_sb[:, cs])
                x = roll.tile([N, TPE], f32, name=f"x{t}", tag=f"x{t}")
                nc.scalar.copy(x[:, :], xb[:, :])          # x = round(x0)
                ps = psp.tile([N, TPE], f32, name=f"psi{t}", tag=f"ps{t}")
                apply_mms(ps, a1_sb[t], xb)
                # hoisted: everything except the PSUM merge
                nc.vector.tensor_scalar_mul(wa_t[t][:, :], x[:, :], -1.0)
                x2 = roll.tile([N, TPE], f32, name=f"x2{t}", tag=f"x2{t}")
                nc.scalar.square(x2[:, :], x[:, :])
                x3 = roll.tile([N, TPE], f32, name=f"x3{t}", tag=f"x3{t}")
                nc.vector.tensor_mul(x3[:, :], x2[:, :], x[:, :])
                dt_ = roll.tile([N, TPE], f32, name=f"dt{t}", tag=f"dt{t}")
                nc.vector.scalar_tensor_tensor(
                    dt_[:, :], x2[:, :], 3.0, da_sb[:, cs],
                    op0=op.mult, op1=op.add,
                )
                nc.vector.reciprocal(r_t[t][:, :], dt_[:, :])
                nc.vector.tensor_mul(s_t[t][:, :], r_t[t][:, :], om_sb[:, cs])
                nc.vector.tensor_sub(F_t[t][:, :], x3[:, :], b_sb[:, cs])
                # PSUM merge + v_1, per column-half for pipelining
                v_bf = roll.tile([N, TPE], bf16, name=f"vb{t}", tag=f"vb{t}")
                for h in range(NHALF):
                    hs = slice(h * HTPE, (h + 1) * HTPE)
                    nc.vector.tensor_add(
                        F_t[t][:, hs], F_t[t][:, hs], ps[:, hs]
                    )
                    nc.vector.tensor_mul(
                        v_bf[:, hs], F_t[t][:, hs], s_t[t][:, hs]
                    )
                x_t[t], x3_t[t], vb_t[t] = x, x3, v_bf

            # ---- one iteration (last one per tile needs no apply) ----
            def emit_iter(k, t):
                if True:
                    corr = k in CORR_T[t]
                    cs = slice(t * TPE, (t + 1) * TPE)
                    x, x3, v_bf = x_t[t], x3_t[t], vb_t[t]
                    F, wa, r, s = F_t[t], wa_t[t], r_t[t], s_t[t]

                    ps = psp.tile([N, TPE], f32, name=f"psk{t}_{k}", tag=f"ps{t}")
                    ps2 = None
                    w32 = w_bf = None
                    if corr:
                        # w-chain first so the A2 matmuls aren't starved
                        w32 = roll.tile([N, TPE], f32, name=f"w{t}_{k}", tag=f"w{t}")
                        nc.vector.tensor_add(w32[:, :], wa[:, :], v_bf[:, :])
                        w_bf = roll.tile(
                            [N, TPE], bf16, name=f"wb{t}_{k}", tag=f"wb{t}"
                        )
                        nc.scalar.copy(w_bf[:, :], w32[:, :])
                        ps2 = psp.tile(
                            [N, TPE], f32, name=f"psc{t}_{k}", tag=f"pc{t}"
                        )
                    apply_mms(ps, a1_sb[t], v_bf)
                    if corr:
                        apply_mms(ps2, a2_sb[t], w_bf)

                    # --- hoisted pointwise (runs under the PE stream) ---
                    xn = roll.tile([N, TPE], f32, name=f"x{t}_{k}", tag=f"x{t}")
                    nc.vector.tensor_sub(xn[:, :], x[:, :], v_bf[:, :])
                    x2 = roll.tile([N, TPE], f32, name=f"x2{t}_{k}", tag=f"x2{t}")
                    nc.scalar.square(x2[:, :], xn[:, :])
                    nx3 = roll.tile([N, TPE], f32, name=f"x3{t}_{k}", tag=f"x3{t}")
                    nc.vector.tensor_mul(nx3[:, :], x2[:, :], xn[:, :])
                    dc = roll.tile([N, TPE], f32, name=f"dc{t}_{k}", tag=f"dt{t}")
                    nc.vector.tensor_sub(dc[:, :], nx3[:, :], x3[:, :])
                    nc.vector.tensor_add(F[:, :], F[:, :], dc[:, :])
                    if corr:
                        nc.vector.tensor_sub(wa[:, :], w32[:, :], w_bf[:, :])
                    else:
                        nc.gpsimd.tensor_add(wa[:, :], wa[:, :], v_bf[:, :])
                    # d~(x'), reciprocal policy, s
                    if k + 1 <= RECIP_NEWTON:
                        dt_ = roll.tile(
                            [N, TPE], f32, name=f"dt{t}_{k}", tag=f"dt{t}"
                        )
                        nc.vector.scalar_tensor_tensor(
                            dt_[:, :], x2[:, :], 3.0, da_sb[:, cs],
                            op0=op.mult, op1=op.add,
                        )
                        if k + 1 <= RECIP_FULL:
                            nc.vector.reciprocal(r[:, :], dt_[:, :])
                        else:
                            # r <- r*(2 - d*r)
                            tmp = roll.tile(
                                [N, TPE], f32, name=f"tm{t}_{k}", tag=f"tm{t}"
                            )
                            nc.vector.tensor_mul(tmp[:, :], dt_[:, :], r[:, :])
                            nc.vector.tensor_scalar(
                                tmp[:, :], tmp[:, :], -1.0, 2.0,
                                op0=op.mult, op1=op.add,
                            )
                            nc.vector.tensor_mul(r[:, :], r[:, :], tmp[:, :])
                        nc.vector.tensor_mul(s[:, :], r[:, :], om_sb[:, cs])

                    # --- PSUM critical path, pipelined per column-slice ---
                    # (quarters when the other tile has retired: less PE work
                    # per iteration to hide the chain under)
                    nsplit = 4 if k > min(NAPPLY_T) else NHALF
                    stpe = TPE // nsplit
                    vbn = roll.tile([N, TPE], bf16, name=f"vb{t}_{k}", tag=f"vb{t}")
                    for h in range(nsplit):
                        hs = slice(h * stpe, (h + 1) * stpe)
                        nc.vector.tensor_sub(F[:, hs], F[:, hs], ps[:, hs])
                        if ps2 is not None:
                            nc.vector.tensor_sub(F[:, hs], F[:, hs], ps2[:, hs])
                        nc.vector.tensor_mul(vbn[:, hs], F[:, hs], s[:, hs])

                    x_t[t], x3_t[t], vb_t[t] = xn, nx3, vbn

            # ---- final half-step + output ----
            def emit_final(t):
                cs = slice(t * TPE, (t + 1) * TPE)
                xn = roll.tile([N, TPE], f32, name=f"xf{t}", tag=f"x{t}")
                nc.vector.tensor_sub(xn[:, :], x_t[t][:, :], vb_t[t][:, :])
                nc.sync.dma_start(out_d[:, cs], xn[:, :])

            # Staggered emission: PE executes in strict program order, so
            # tile1 (whose weights arrive later) trails tile0 by one unit to
            # avoid head-of-line blocking during the load phase.
            units = {
                t: (
                    [("init", t)]
                    + [("iter", k, t) for k in range(1, NAPPLY_T[t] + 1)]
                    + [("final", t)]
                )
                for t in range(NTILES)
            }
            seq = []
            n0, n1 = len(units[0]), len(units[1])
            for i in range(max(n0, n1 + 1)):
                if i < n0:
                    seq.append(units[0][i])
                if 0 <= i - 1 < n1:
                    seq.append(units[1][i - 1])
            for u in seq:
                if u[0] == "init":
                    emit_init(u[1])
                elif u[0] == "iter":
                    emit_iter(u[1], u[2])
                else:
                    emit_final(u[1])

    nc.compile()
    return nc


def _get_compiled():
    global _compiled
    if _compiled is None:
        _compiled = _build()
    return _compiled


def _perm_for(omega):
    """Global omega sort: slow (low omega) half feeds every core's tile0,
    fast half feeds tile1. perm[slot] = source batch index."""
    order = np.argsort(np.asarray(omega, dtype=np.float32)[:, 0], kind="stable")
    half = BATCH // 2
    perm = np.empty(BATCH, dtype=np.int64)
    for c in range(NCORES):
        perm[c * PER_CORE : c * PER_CORE + TPE] = order[c * TPE : (c + 1) * TPE]
        perm[c * PER_CORE + TPE : (c + 1) * PER_CORE] = order[
            half + c * TPE : half + (c + 1) * TPE
        ]
    return perm


def _prep_inputs(x, A, b, omega, perm):
    """Host-side shard + layout prep. Returns list of per-core in_maps."""
    A = np.ascontiguousarray(A, dtype=np.float32)
    x = np.asarray(x, dtype=np.float32)[perm]
    b = np.asarray(b, dtype=np.float32)[perm]
    omega = np.asarray(omega, dtype=np.float32)[perm]

    Ap = A[perm]
    A1 = Ap.astype(_BF16)
    A2 = (Ap - A1.astype(np.float32)).astype(_BF16)
    dA = np.ascontiguousarray(np.diagonal(Ap, axis1=1, axis2=2))

    in_maps = []
    for c in range(NCORES):
        sl = slice(c * PER_CORE, (c + 1) * PER_CORE)
        m = {}
        for t in range(NTILES):
            ts = slice(c * PER_CORE + t * TPE, c * PER_CORE + (t + 1) * TPE)
            # lhsT layout [j, (e, i)]: element e's weights = A[e].T
            m[f"at1_{t}"] = np.ascontiguousarray(
                A1[ts].transpose(2, 0, 1)
            ).reshape(N, TPE * N)
            m[f"at2_{t}"] = np.ascontiguousarray(
                A2[ts].transpose(2, 0, 1)
            ).reshape(N, TPE * N)
        m["x0t"] = np.ascontiguousarray(x[sl].T)
        m["bt"] = np.ascontiguousarray(b[sl].T)
        m["dat"] = np.ascontiguousarray(dA[sl].T)
        m["omt"] = np.ascontiguousarray(
            np.broadcast_to(omega[sl].reshape(1, PER_CORE), (N, PER_CORE))
        )
        in_maps.append(m)
    return in_maps


def _run(inputs, trace=False):
    from concourse.bass_utils import run_bass_kernel_spmd

    nc = _get_compiled()
    perm = _perm_for(inputs["omega"])
    in_maps = _prep_inputs(
        inputs["x"], inputs["A"], inputs["b"], inputs["omega"], perm
    )
    res = run_bass_kernel_spmd(
        nc, in_maps, core_ids=list(range(NCORES)), trace=trace
    )
    out = np.empty((BATCH, N), dtype=np.float32)
    for c in range(NCORES):
        out[perm[c * PER_CORE : (c + 1) * PER_CORE]] = res.results[c]["outt"].T
    return out, res


def kernel(x, A, b, omega):
    out, _ = _run({"x": x, "A": A, "b": b, "omega": omega}, trace=False)
    return out
